# revision 1
# baseline (speedup 1.0000x reference)
"""Trainium2 Bass kernel for nn_BinaryTreeShInvariantConv.

Per (b, v): gather P=32 neighbor rows of signal[b] (Cin=64), contract over P
against conv_kernel[b,v] -> y[Cin, R*N], square, sum SH orders per degree l,
sqrt(+eps), contract [Cin*R*(L+1)=512] against kernel_weights -> [Cout=128],
bias + relu.

Sharding: data-parallel over batch B=8 -> one batch per NeuronCore (SPMD).

Dataflow per core (one batch, V=4096), in supergroups of 128 v's:
  - dma_gather: 4096 rows (128 v x 32 p) of bf16-padded signal -> patches
    [128 part=(v4,p), 32 chunk, 128 ch] (only ch 0:64 real).
  - Kbd: block-diagonal conv_kernel in "j-strided" layout [128, (j,g,rn)]:
    off-diagonal zeros memset once and persistent; diagonal refilled each
    supergroup by 4 contiguous DMAs from host-pretransposed K_re.
  - MM1 per 4-v chunk: lhsT = patches chunk [128,64], rhs = Kbd strided AP
    [128, (j 4, rn 32)] -> psum [64 c, 128 (v4,rn)]; chunk pairs col-tiled
    into 128 partitions; 8 chunks per PSUM bank [128, 512].
  - square (ACT x3 banks, DVE x1 bank) -> ysq bf16 [128, 2048].
  - degree-sum: 4x DVE reduce_sum over n-windows (sizes 1,3,5,7) -> zpre f32.
  - sqrt(x + 1e-4) on ACT -> zsb bf16 (eps-add approximates max(x,eps);
    x >= 0 so error is bounded and negligible after the final contraction).
  - MM3: per (r,l) and per half: lhsT = zsb slice [64 c, 64 v], rhs = W
    [64 c, 128 i], accumulate 8 slices in PSUM -> [128 v, 128 i].
  - relu (+ bias if nonzero) on ACT -> out.
"""

import sys

sys.path.insert(0, "/opt/trn_rl_repo")

import numpy as np

import concourse.bacc as bacc
import concourse.mybir as mybir
import concourse.tile as tile
from concourse import bass2jax

B, V, P, CIN, R, COUT = 8, 4096, 32, 64, 2, 128
NSH, NDEG = 16, 4
VSG = 128            # v's per supergroup
NSG = V // VSG       # 32 supergroups
NCHUNK = VSG // 4    # 32 chunks of 4 v's
SGI = VSG * P        # 4096 gather indices per supergroup
GSZ = 4096           # indices per dma_gather op (single_packet=False
                     # streams packets, so ops may exceed the SWDGE ring)
BF16 = mybir.dt.bfloat16
F32 = mybir.dt.float32
I16 = mybir.dt.int16

_CACHE = {}
_SKIP = set()  # debug: subset of {'gather','mm1','post','mm3','store'}


def _dma_gather_any(eng, out_ap, in_ap, idxs_ap, num_idxs, elem_size,
                    single_packet=True):
    """bass.dma_gather minus the elem_size%256 assert (the Q7 ucode only
    requires the source ROW STRIDE to be a 256B multiple; the bytes read per
    row are free). in_ap's outer stride (elem_step) must be 256B-aligned."""
    from concourse import ap_utils
    from concourse.bass import MemorySpace

    assert idxs_ap.dtype == I16
    assert in_ap.space == MemorySpace.DRAM
    assert in_ap.dtype == out_ap.dtype
    elem_step = in_ap.ap[0][0]
    stride_bytes = elem_step * mybir.dt.size(in_ap.dtype)
    assert stride_bytes % 256 == 0 and stride_bytes // 256 < 256
    assert ap_utils.ap_is_contiguous(out_ap.ap[1:])
    assert ap_utils.ap_is_contiguous(idxs_ap.ap[1:])
    assert in_ap.ap[-1][1] == out_ap.ap[-1][1] == elem_size
    assert out_ap.ap[0][1] * out_ap.ap[1][1] == ((num_idxs + 127) // 128) * 128

    _in_ap = eng.lower_ap_dma(in_ap, for_custom_bir_dma=True)
    return eng.add_instruction(
        mybir.InstDMAGatherAnt(
            name=eng.bass.get_next_instruction_name(),
            ins=[*_in_ap, eng.lower_ap(idxs_ap),
                 eng.lower_val_access(eng.to_reg(num_idxs))],
            outs=[eng.lower_ap(out_ap)],
            transpose=False,
            num_idxs=num_idxs,
            elem_size=elem_size,
            stride_bytes_256=stride_bytes // 256,
            gen_mode=0,
            single_packet=single_packet,
            queue_num=0,
            sbuf_tokens_per_rank=0,
            sbuf_free_dim_per_rank=0,
            sbuf_free_dim_pad_per_rank=0,
            sbuf_byte_offset=0,
        ))


def _build_nc(nsg, with_bias):
    nc = bacc.Bacc("TRN2", target_bir_lowering=False, debug=False,
                   enable_asserts=False)
    vtot = nsg * VSG
    sig = nc.dram_tensor("sig", [V, 128], BF16, kind="ExternalInput")
    kre = nc.dram_tensor("kre", [nsg, 4, P, NCHUNK, R * NSH], BF16,
                         kind="ExternalInput")
    idx = nc.dram_tensor("idx", [128, (SGI // 16) * nsg], I16,
                         kind="ExternalInput")
    wsb = nc.dram_tensor("wsb", [128, 8 * COUT], BF16, kind="ExternalInput")
    bia = nc.dram_tensor("bia", [1, COUT], F32, kind="ExternalInput")
    outd = nc.dram_tensor("outd", [vtot, COUT], F32, kind="ExternalOutput")

    AF = mybir.ActivationFunctionType
    with tile.TileContext(nc) as tc:
        with (
            tc.tile_pool(name="const", bufs=1) as constp,
            tc.tile_pool(name="kbd", bufs=2) as kbdp,
            tc.tile_pool(name="patches", bufs=3) as patp,
            tc.tile_pool(name="ysq", bufs=3) as ysqp,
            tc.tile_pool(name="zpre", bufs=3) as zprep,
            tc.tile_pool(name="zsb", bufs=3) as zsbp,
            tc.tile_pool(name="osb", bufs=3) as osbp,
            tc.tile_pool(name="ps1", bufs=3, space="PSUM") as ps1p,
            tc.tile_pool(name="ps3", bufs=2, space="PSUM") as ps3p,
        ):
            w_t = constp.tile([128, 8 * COUT], BF16, tag="w")
            nc.sync.dma_start(w_t[:], wsb.ap())
            idx_t = constp.tile([128, (SGI // 16) * nsg], I16, tag="idx")
            nc.sync.dma_start(idx_t[:], idx.ap())
            if with_bias:
                bias_t = constp.tile([1, COUT], F32, tag="bias")
                nc.sync.dma_start(bias_t[:], bia.ap())

            eps_t = constp.tile([128, 1], F32, tag="eps")
            nc.vector.memset(eps_t[:], 1e-4)

            kbds = [kbdp.tile([128, 4 * 1024], BF16, tag="kbd", name=f"kbd{i}") for i in range(2)]
            nc.vector.memset(kbds[0][:], 0.0)
            nc.vector.memset(kbds[1][:], 0.0)

            for sg in range(nsg):
                # --- gather patches ---------------------------------------
                pat = patp.tile([128, NCHUNK, CIN], BF16, tag="pat")
                ngo = SGI // GSZ    # gather ops per supergroup
                pat_r = pat[:, :, :].rearrange("p (t u) c -> p t u c", t=ngo)
                for t in range(ngo if 'gather' not in _SKIP else 0):
                    _dma_gather_any(
                        nc.gpsimd, pat_r[:, t], sig.ap()[:, 0:CIN],
                        idx_t[:, (SGI // 16) * sg + (GSZ // 16) * t:
                              (SGI // 16) * sg + (GSZ // 16) * (t + 1)],
                        GSZ, CIN, single_packet=False)

                # --- fill Kbd diagonal ------------------------------------
                kbd = kbds[sg % 2]
                for j in range(4):
                    nc.sync.dma_start(
                        kbd[32 * j:32 * (j + 1), 1024 * j:1024 * (j + 1)],
                        kre.ap()[sg, j])

                kbd_j = kbd[:, :].rearrange("p (j w) -> p j w", j=4)

                # --- MM1: per-chunk conv over p ---------------------------
                ps1 = [ps1p.tile([128, 1024], F32, tag="ps1", name=f"ps1_{q}") for q in range(2)]
                for g in range(NCHUNK if 'mm1' not in _SKIP else 0):
                    bank, blk, half = g // 16, (g % 16) // 2, g % 2
                    lhsT = pat[:, g, :]
                    rhs = kbd_j[:, :, 32 * g:32 * (g + 1)]
                    out = ps1[bank][64 * half:64 * (half + 1),
                                    128 * blk:128 * (blk + 1)]
                    nc.tensor.matmul(out, lhsT, rhs, start=True, stop=True)

                # --- square: 3 banks on ACT, 1 on DVE ---------------------
                ysq = ysqp.tile([128, 2048], BF16, tag="ysq")
                for q in range(2 if 'post' not in _SKIP else 0):
                    dst = ysq[:, 1024 * q:1024 * (q + 1)]
                    nc.scalar.activation(dst, ps1[q][:], AF.Square)

                # --- degree sums over n-windows ---------------------------
                zpre = zprep.tile([128, 512], F32, tag="zpre")
                ysq_r = ysq[:, :].rearrange("p (a b n) -> p a b n", a=16, b=8)
                zpre_r = zpre[:, :].rearrange("p (a b l) -> p a b l", a=16, b=8)
                for l in range(NDEG if 'post' not in _SKIP else 0):
                    w_l = 2 * l + 1
                    nc.vector.reduce_sum(
                        zpre_r[:, :, :, l],
                        ysq_r[:, :, :, l * l:l * l + w_l],
                        axis=mybir.AxisListType.X)

                # --- sqrt(x + eps) ----------------------------------------
                zsb = zsbp.tile([128, 512], BF16, tag="zsb")
                if 'post' not in _SKIP:
                    nc.scalar.activation(zsb[:], zpre[:], AF.Sqrt, bias=eps_t[:])

                # --- MM3: contract (c, r, l) against W --------------------
                ps3 = ps3p.tile([128, COUT], F32, tag="ps3")
                zsb_r = zsb[:, :].rearrange("p (a v rl) -> p a v rl", a=16, v=4)
                for half in range(2 if 'mm3' not in _SKIP else 0):
                    for rl in range(8):
                        pa, pb = 64 * half, 64 * (half + 1)
                        lhsT = zsb_r[pa:pb, :, :, rl]
                        rhs = w_t[pa:pb, COUT * rl:COUT * (rl + 1)]
                        nc.tensor.matmul(ps3[pa:pb, :], lhsT, rhs,
                                         start=(rl == 0), stop=(rl == 7),
                                         skip_group_check=True)

                # --- bias + relu + store ----------------------------------
                osb = osbp.tile([128, COUT], F32, tag="osb")
                if with_bias:
                    nc.vector.tensor_add(
                        osb[:], ps3[:],
                        bias_t[:, :].broadcast(0, 128))
                    nc.scalar.activation(osb[:], osb[:], AF.Relu)
                else:
                    nc.vector.tensor_scalar_max(osb[:], ps3[:], 0.0)
                dst = outd.ap()[VSG * sg:VSG * (sg + 1), :].rearrange(
                    "(q h v) i -> h q v i", q=16, h=2, v=4)
                nc.sync.dma_start(dst[0], osb[0:64, :])
                nc.sync.dma_start(dst[1], osb[64:128, :])

    nc.compile()
    return nc


def _prep_inputs_core(b, signal, patches_idx, conv_kernel, kernel_weights,
                      biases, nsg):
    bf = mybir.dt.np(BF16)
    sig = np.zeros((V, 128), dtype=bf)
    sig[:, :CIN] = signal[b].astype(bf)
    # K_re[sg, j, p, g, rn] = conv_kernel[b, 128*sg + 4*g + j, p, rn]
    k = conv_kernel[b].reshape(nsg, NCHUNK, 4, P, R * NSH)
    kre = np.ascontiguousarray(k.transpose(0, 2, 3, 1, 4)).astype(bf)
    # wsb[c + 64*dup, rl*128 + i] = kernel_weights[i, c, r, l]
    w = kernel_weights.transpose(2, 3, 1, 0).reshape(8, CIN, COUT)
    wrow = np.ascontiguousarray(w.transpose(1, 0, 2)).reshape(CIN, 8 * COUT)
    wsb = np.concatenate([wrow, wrow], axis=0).astype(bf)
    bia = biases.reshape(1, COUT).astype(np.float32)
    idx = _fix_idx_wrap(patches_idx[b, :, :, 1].astype(np.int16).reshape(-1))
    return {"sig": sig, "kre": kre, "idx": idx, "wsb": wsb, "bia": bia}


def _fix_idx_wrap(pidx_flat):
    # wrap order is per gather op: each op's GSZ idxs wrapped into 16
    # partitions independently.
    blk = pidx_flat.reshape(-1, GSZ // 16, 16)   # [ops, GSZ/16, 16]
    out = np.ascontiguousarray(
        blk.transpose(0, 2, 1).transpose(1, 0, 2)).reshape(16, -1)
    return np.tile(out, (8, 1))


def _make_runner(nc, n_cores=8):
    import jax
    from jax.sharding import Mesh, PartitionSpec
    from jax.experimental.shard_map import shard_map

    bass2jax.install_neuronx_cc_hook()
    partition_name = (nc.partition_id_tensor.name
                      if nc.partition_id_tensor else None)
    in_names, out_names, out_avals, zero_outs = [], [], [], []
    for alloc in nc.m.functions[0].allocations:
        if not isinstance(alloc, mybir.MemoryLocationSet):
            continue
        name = alloc.memorylocations[0].name
        if alloc.kind == "ExternalInput":
            if name != partition_name:
                in_names.append(name)
        elif alloc.kind == "ExternalOutput":
            out_names.append(name)
            shape = tuple(alloc.tensor_shape)
            dtype = mybir.dt.np(alloc.dtype)
            out_avals.append(jax.core.ShapedArray(shape, dtype))
            zero_outs.append(np.zeros(shape, dtype))
    n_params, n_outs = len(in_names), len(out_avals)
    in_names_all = list(in_names) + list(out_names)
    if partition_name is not None:
        in_names_all.append(partition_name)

    def _body(*args):
        operands = list(args)
        if partition_name is not None:
            operands.append(bass2jax.partition_id_tensor())
        outs = bass2jax._bass_exec_p.bind(
            *operands, out_avals=tuple(out_avals),
            in_names=tuple(in_names_all), out_names=tuple(out_names),
            lowering_input_output_aliases=(),
            sim_require_finite=True, sim_require_nnan=True, nc=nc)
        return tuple(outs)

    donate = tuple(range(n_params, n_params + n_outs))
    devices = jax.devices()[:n_cores]
    mesh = Mesh(np.asarray(devices), ("core",))
    sharded = jax.jit(
        shard_map(_body, mesh=mesh,
                  in_specs=(PartitionSpec("core"),) * (n_params + n_outs),
                  out_specs=(PartitionSpec("core"),) * n_outs,
                  check_rep=False),
        donate_argnums=donate, keep_unused=True)

    def run_fn(in_maps):
        import jax
        per_core = [[np.asarray(m[nm]) for nm in in_names] for m in in_maps]
        concat_in = [
            np.concatenate([per_core[c][i] for c in range(n_cores)], axis=0)
            for i in range(n_params)]
        concat_zeros = [
            np.zeros((n_cores * z.shape[0], *z.shape[1:]), z.dtype)
            for z in zero_outs]
        out_arrs = sharded(*concat_in, *concat_zeros)
        jax.block_until_ready(out_arrs)
        return [
            {nm: np.asarray(out_arrs[i]).reshape(n_cores, *out_avals[i].shape)[c]
             for i, nm in enumerate(out_names)}
            for c in range(n_cores)]

    return run_fn


def kernel(signal, patches_idx, conv_kernel, kernel_weights, biases):
    with_bias = bool(np.any(biases))
    key = ("k", NSG, with_bias)
    if key not in _CACHE:
        nc = _build_nc(NSG, with_bias)
        _CACHE[key] = (nc, _make_runner(nc))
    nc, run = _CACHE[key]

    in_maps = []
    for b in range(B):
        m = _prep_inputs_core(b, signal, patches_idx, conv_kernel,
                              kernel_weights, biases, NSG)
        in_maps.append(m)

    results = run(in_maps)
    out = np.stack([results[b]["outd"] for b in range(B)], axis=0)
    return out.astype(np.float32)



# revision 18
# speedup vs baseline: 1.6953x; 1.6953x over previous
"""Trainium2 Bass kernel for nn_BinaryTreeShInvariantConv.

Per (b, v): gather P=32 neighbor rows of signal[b] (Cin=64), contract over P
against conv_kernel[b,v] -> y[Cin, R*N], square, sum SH orders per degree l,
sqrt(+eps), contract [Cin*R*(L+1)=512] against kernel_weights -> [Cout=128],
bias + relu.

Sharding: data-parallel over batch B=8 -> one batch per NeuronCore (SPMD).

Design (driven by the CoreSim v1 cost model, which prices each instruction
as free-size x engine-cycle charged serially to its issuing engine):
  - Gather reads bf16 rows PACKED AS uint64 (16 u64 = 64 bf16 channels):
    the gather is priced as a generic Pool op at out-free-ELEMENTS x 0.83ns,
    so 8x fewer elements -> 4 ops x 427ns per 512-v supergroup (13.7us total
    vs 218us naive).
  - "Pair-diagonal" lhsT: gathered rows land directly in block-diagonal
    [64 part, 128 col] bf16 slabs (2 v's per slab; off-diag zeros memset
    once per buffer). MM1 -> [128 part = (v-parity, c), 32 rn] per pair:
    half the PE columns of a 4-v block-diag rhs; conv_kernel needs one
    [128, 4096] DMA per supergroup.
  - Degree sums as strided bf16 tensor_tensor adds (2x DVE mode) instead of
    reduce_sum (no fast mode).
  - Squares (PSUM f32 -> bf16) split between ACT (activation Square) and
    Pool (tensor_tensor mult) to balance engine occupancy.
  - MM3 packs 128 v-pairs in output partitions.

Supergroup = 512 v's (NSG=8): 4 gather ops (one per lhsT quadrant class,
4096 idxs each), 1 kre DMA, 8x32 MM1 matmuls -> 8 PSUM granules -> squares
-> per 256-v block: DVE degree-adds, ACT sqrt(+eps), MM3 (2 par x 8 rl
accumulating), DVE relu, store.
"""

import sys

sys.path.insert(0, "/opt/trn_rl_repo")

import numpy as np

import concourse.bacc as bacc
import concourse.mybir as mybir
import concourse.tile as tile
from concourse import ap_utils
from concourse.bass import MemorySpace
from concourse import bass2jax

B, V, P, CIN, R, COUT = 8, 4096, 32, 64, 2, 128
NSH, NDEG = 16, 4
VSG = 512            # v's per supergroup
NSG = V // VSG       # 8 supergroups
NSLAB = 128          # v-pairs per part-block (h) per supergroup
NPOOL_SQ = 3         # squares per supergroup done on Pool (rest on ACT)
BF16 = mybir.dt.bfloat16
F32 = mybir.dt.float32
I16 = mybir.dt.int16
U64 = mybir.dt.uint64

_CACHE = {}
_SKIP = set()  # debug: subset of {'gather','mm1','post','mm3','store'}


def _dma_gather_any(eng, out_ap, in_ap, idxs_ap, num_idxs, elem_size,
                    single_packet=True, nreg=None):
    """bass.dma_gather with relaxed asserts. The executor flattens the out AP
    and reshapes it to (128, ceil(n/128), elem) as a VIEW: row i lands at
    flat slot (i%128)*chunks + i//128. The AP's stride tree must therefore be
    numpy-reshape-compatible with that shape (seamless merges)."""
    assert idxs_ap.dtype == I16
    assert in_ap.space == MemorySpace.DRAM
    assert in_ap.dtype == out_ap.dtype
    elem_step = in_ap.ap[0][0]
    stride_bytes = elem_step * mybir.dt.size(in_ap.dtype)
    assert stride_bytes % 256 == 0 and stride_bytes // 256 < 256
    assert ap_utils.ap_is_contiguous(idxs_ap.ap[1:])
    assert in_ap.ap[-1][1] == elem_size
    total = 1
    for _, c in out_ap.ap:
        total *= c
    assert total == ((num_idxs + 127) // 128) * 128 * elem_size

    _in_ap = eng.lower_ap_dma(in_ap, for_custom_bir_dma=True)
    if nreg is None:
        nreg = eng.to_reg(num_idxs)
    return eng.add_instruction(
        mybir.InstDMAGatherAnt(
            name=eng.bass.get_next_instruction_name(),
            ins=[*_in_ap, eng.lower_ap(idxs_ap),
                 eng.lower_val_access(nreg)],
            outs=[eng.lower_ap(out_ap)],
            transpose=False,
            num_idxs=num_idxs,
            elem_size=elem_size,
            stride_bytes_256=stride_bytes // 256,
            gen_mode=0,
            single_packet=single_packet,
            queue_num=0,
            sbuf_tokens_per_rank=0,
            sbuf_free_dim_per_rank=0,
            sbuf_free_dim_pad_per_rank=0,
            sbuf_byte_offset=0,
        ))


def _build_nc(with_bias):
    nc = bacc.Bacc("TRN2", target_bir_lowering=False, debug=False,
                   enable_asserts=False, dynamic_dma_scratch_size=32768)
    AF = mybir.ActivationFunctionType
    ALU = mybir.AluOpType

    sig = nc.dram_tensor("sig", [V, 32], U64, kind="ExternalInput")
    kre = nc.dram_tensor("kre", [NSG, 128, NSLAB, 2 * NSH], BF16,
                         kind="ExternalInput")
    idx = nc.dram_tensor("idx", [16, V * P // 16], I16, kind="ExternalInput")
    wsb = nc.dram_tensor("wsb", [128, 8 * COUT], BF16, kind="ExternalInput")
    bia = nc.dram_tensor("bia", [1, COUT], F32, kind="ExternalInput")
    outd = nc.dram_tensor("outd", [V, COUT], F32, kind="ExternalOutput")

    with tile.TileContext(nc) as tc:
        with (
            tc.tile_pool(name="const", bufs=1) as constp,
            tc.tile_pool(name="patbf", bufs=2) as patbfp,
            tc.tile_pool(name="kre", bufs=2) as krep,
            tc.tile_pool(name="ysq", bufs=3) as ysqp,
            tc.tile_pool(name="zsb", bufs=2) as zsbp,
            tc.tile_pool(name="zt", bufs=2) as ztp,
            tc.tile_pool(name="osb", bufs=2) as osbp,
            tc.tile_pool(name="ps1", bufs=3, space="PSUM") as ps1p,
            tc.tile_pool(name="ps3", bufs=2, space="PSUM") as ps3p,
        ):
            w_t = constp.tile([128, 8 * COUT], BF16, tag="w")
            nc.sync.dma_start(w_t[:], wsb.ap())
            idx_t = constp.tile([128, V * P // 16], I16, tag="idx")
            nc.vector.memset(idx_t[:, :].bitcast(F32), 0.0)
            nc.sync.dma_start(idx_t[0:16, :], idx.ap())
            if with_bias:
                bias_t = constp.tile([1, COUT], F32, tag="bias")
                nc.sync.dma_start(bias_t[:], bia.ap())
            eps_t = constp.tile([128, 1], F32, tag="eps")
            nc.vector.memset(eps_t[:], 1e-4)

            # bf16 patch tiles: zero quadrants persist across supergroups;
            # gathers refill only the data quadrants. Memset buffer 0 on Pool
            # and buffer 1 on DVE so startup overlaps.
            patbfs = [patbfp.tile([128, NSLAB * 128], BF16, tag="patbf",
                                  name=f"patbf_{i}") for i in range(2)]
            nc.gpsimd.memset(patbfs[0][:, :].bitcast(F32), 0.0)
            nc.vector.memset(patbfs[1][:, :].bitcast(F32), 0.0)

            nreg = nc.gpsimd.to_reg(VSG * P // 4)

            for sg in range(NSG):
                sgb = VSG * sg
                # --- gathers: 4 quadrant ops, 4096 idxs each ---------------
                patbf = patbfs[sg % 2]
                # u64 view: [p][b 32][e 4][blk 2][c8 16]; slab s=32*(i%4)+i//128
                p8v = patbf[:, :].bitcast(U64).rearrange(
                    "p (bb e blk c) -> p bb e blk c", bb=32, e=4, blk=2)
                for op in range(4 if 'gather' not in _SKIP else 0):
                    h, par = op // 2, op % 2
                    gv = p8v[64 * h + 32 * par:64 * h + 32 * par + 32,
                             :, :, par, :]
                    col0 = (V * P // 16) // NSG * sg + 256 * op
                    _dma_gather_any(
                        nc.gpsimd, gv, sig.ap()[:, 0:CIN // 4],
                        idx_t[:, col0:col0 + 256], VSG * P // 4, CIN // 4,
                        single_packet=False, nreg=nreg)

                # --- kre load ----------------------------------------------
                kre_t = krep.tile([128, NSLAB * 2 * NSH], BF16, tag="kre")
                nc.sync.dma_start(kre_t[:], kre.ap()[sg])

                # --- per part-block h: MM1 granules + post + MM3 -----------
                for h in range(2):
                    ysq = ysqp.tile([128, 4 * 32 * 32], BF16, tag="ysq")
                    for gl in range(4):
                        ps1 = ps1p.tile([128, 1024], F32, tag="ps1")
                        for ql in range(32 if 'mm1' not in _SKIP else 0):
                            s = 32 * gl + ql
                            nc.tensor.matmul(
                                ps1[:, 32 * ql:32 * ql + 32],
                                patbf[64 * h:64 * h + 64,
                                      128 * s:128 * s + 128],
                                kre_t[64 * h:64 * h + 64,
                                      32 * s:32 * s + 32],
                                start=True, stop=True)
                        if 'post' not in _SKIP:
                            dst = ysq[:, 1024 * gl:1024 * (gl + 1)]
                            if 4 * h + gl < NPOOL_SQ:
                                nc.gpsimd.tensor_tensor(dst, ps1[:], ps1[:],
                                                        ALU.mult)
                            else:
                                nc.scalar.activation(dst, ps1[:], AF.Square)

                    # ysq: [128, (g 4, pl 32, n 16, r 2)]
                    yv = ysq[:, :].rearrange("p (g pl n r) -> p g pl n r",
                                             g=4, pl=32, n=NSH)
                    # n split (n2, two): n = 2*n2 + two, for stride-2 windows
                    yv2 = ysq[:, :].rearrange(
                        "p (g pl n2 two r) -> p g pl n2 two r",
                        g=4, pl=32, n2=NSH // 2, two=2)
                    # zsb: [128, (g 4, pl 32, l 4, r 2)]
                    zsb = zsbp.tile([128, 4 * 32 * NDEG * 2], BF16, tag="zsb")
                    zv = zsb[:, :].rearrange("p (g pl l r) -> p g pl l r",
                                             g=4, pl=32, l=NDEG)
                    # zt scratch: [128, (g 4, pl 32, t 5, r 2)]
                    zt = ztp.tile([128, 4 * 32 * 5 * 2], BF16, tag="zt")
                    tv = zt[:, :].rearrange("p (g pl t r) -> p g pl t r",
                                            g=4, pl=32, t=5)
                    TT = nc.vector.tensor_tensor
                    if 'post' not in _SKIP:
                        # l=3: n 9..15 (7 terms): {9,11,13}+{10,12,14}, +15
                        TT(tv[:, :, :, 0:3, :], yv2[:, :, :, 4:7, 1, :],
                           yv2[:, :, :, 5:8, 0, :], ALU.add)
                        TT(zv[:, :, :, 3, :], tv[:, :, :, 0, :],
                           tv[:, :, :, 1, :], ALU.add)
                        TT(zv[:, :, :, 3, :], zv[:, :, :, 3, :],
                           tv[:, :, :, 2, :], ALU.add)
                        TT(zv[:, :, :, 3, :], zv[:, :, :, 3, :],
                           yv2[:, :, :, 7, 1, :], ALU.add)
                        # l=2: n 4..8 (5 terms): {4,6}+{5,7}, +8
                        TT(tv[:, :, :, 3:5, :], yv2[:, :, :, 2:4, 0, :],
                           yv2[:, :, :, 2:4, 1, :], ALU.add)
                        TT(zv[:, :, :, 2, :], tv[:, :, :, 3, :],
                           tv[:, :, :, 4, :], ALU.add)
                        TT(zv[:, :, :, 2, :], zv[:, :, :, 2, :],
                           yv2[:, :, :, 4, 0, :], ALU.add)
                        # l=1: n 1..3
                        TT(zv[:, :, :, 1, :], yv2[:, :, :, 0, 1, :],
                           yv2[:, :, :, 1, 0, :], ALU.add)
                        TT(zv[:, :, :, 1, :], zv[:, :, :, 1, :],
                           yv2[:, :, :, 1, 1, :], ALU.add)
                        # sqrt(x + eps): l=0 from ysq n=0; l>=1 in place
                        nc.scalar.activation(zv[:, :, :, 0, :],
                                             yv[:, :, :, 0, :], AF.Sqrt,
                                             bias=eps_t[:])
                        nc.scalar.activation(zv[:, :, :, 1:4, :],
                                             zv[:, :, :, 1:4, :], AF.Sqrt,
                                             bias=eps_t[:])

                    # --- MM3: out[pair, (par, i)] --------------------------
                    ps3 = ps3p.tile([128, 2 * COUT], F32, tag="ps3")
                    for par in range(2 if 'mm3' not in _SKIP else 0):
                        for rl in range(8):
                            lhsT = zv[64 * par:64 * par + 64, :, :, rl // 2,
                                      rl % 2]
                            rhs = w_t[64 * par:64 * par + 64,
                                      COUT * rl:COUT * (rl + 1)]
                            nc.tensor.matmul(
                                ps3[:, COUT * par:COUT * (par + 1)],
                                lhsT, rhs, start=(rl == 0), stop=(rl == 7),
                                skip_group_check=True)

                    # --- relu (+bias) + store ------------------------------
                    osb = osbp.tile([128, 2 * COUT], F32, tag="osb")
                    if with_bias:
                        for par in range(2):
                            nc.vector.tensor_add(
                                osb[:, COUT * par:COUT * (par + 1)],
                                ps3[:, COUT * par:COUT * (par + 1)],
                                bias_t[:, :].broadcast(0, 128))
                        nc.scalar.activation(osb[:], osb[:], AF.Relu)
                    else:
                        nc.vector.tensor_scalar_max(osb[:], ps3[:], 0.0)
                    if 'store' not in _SKIP:
                        dst = outd.ap()[sgb + 256 * h:sgb + 256 * (h + 1), :]
                        dst = dst.rearrange("(pl par) i -> pl par i", par=2)
                        nc.sync.dma_start(
                            dst, osb[:, :].rearrange("p (par i) -> p par i",
                                                     par=2))

    nc.compile()
    return nc


# map (p_pt, s) -> gather list position i (see _dma_gather_any docstring)
_SS, _PP = np.meshgrid(np.arange(NSLAB), np.arange(32), indexing='ij')
_IPOS = (128 * (_SS % 32) + 4 * _PP + _SS // 32).ravel()


def _prep_inputs_core(b, signal, patches_idx, conv_kernel, kernel_weights,
                      biases):
    bf = mybir.dt.np(BF16)
    sigrow = np.zeros((V, 128), dtype=bf)
    sigrow[:, 0:CIN] = signal[b].astype(bf)
    sig_u64 = np.ascontiguousarray(sigrow).view(np.uint64)   # [V, 32]

    pidx = patches_idx[b, :, :, 1]
    krn = conv_kernel[b].transpose(0, 1, 3, 2)              # [V, P, NSH, R]
    # kre[sg, 64h+32par+p, s, n*2+r] = krn[sg*512+256h+2s+par, p]
    k6 = krn.reshape(NSG, 2, NSLAB, 2, P, 2 * NSH)
    kre_ = np.ascontiguousarray(k6.transpose(0, 1, 3, 4, 2, 5)).reshape(
        NSG, 128, NSLAB, 2 * NSH).astype(bf)

    # idx: per sg, 4 ops x 4096 idxs; arr[i] = pidx[v(h,par,s), p_pt]
    idxh = np.empty((16, V * P // 16), dtype=np.int16)
    for sg in range(NSG):
        for op in range(4):
            h, par = op // 2, op % 2
            vv = VSG * sg + 256 * h + 2 * _SS + par
            mat = pidx[vv.ravel(), _PP.ravel()].astype(np.int16)
            arr = np.empty(VSG * P // 4, dtype=np.int16)
            arr[_IPOS] = mat
            col0 = (V * P // 16) // NSG * sg + 256 * op
            idxh[:, col0:col0 + 256] = arr.reshape(256, 16).T

    w = kernel_weights.transpose(1, 3, 2, 0).reshape(CIN, 8 * COUT)
    wsb = np.concatenate([w, w], axis=0).astype(bf)
    bia = biases.reshape(1, COUT).astype(np.float32)
    return {"sig": sig_u64, "kre": kre_, "idx": idxh, "wsb": wsb, "bia": bia}


def _make_runner(nc, n_cores=8):
    import jax
    from jax.sharding import Mesh, PartitionSpec
    from jax.experimental.shard_map import shard_map

    bass2jax.install_neuronx_cc_hook()
    partition_name = (nc.partition_id_tensor.name
                      if nc.partition_id_tensor else None)
    in_names, out_names, out_avals, zero_outs = [], [], [], []
    for alloc in nc.m.functions[0].allocations:
        if not isinstance(alloc, mybir.MemoryLocationSet):
            continue
        name = alloc.memorylocations[0].name
        if alloc.kind == "ExternalInput":
            if name != partition_name:
                in_names.append(name)
        elif alloc.kind == "ExternalOutput":
            out_names.append(name)
            shape = tuple(alloc.tensor_shape)
            dtype = mybir.dt.np(alloc.dtype)
            out_avals.append(jax.core.ShapedArray(shape, dtype))
            zero_outs.append(np.zeros(shape, dtype))
    n_params, n_outs = len(in_names), len(out_avals)
    in_names_all = list(in_names) + list(out_names)
    if partition_name is not None:
        in_names_all.append(partition_name)

    def _body(*args):
        operands = list(args)
        if partition_name is not None:
            operands.append(bass2jax.partition_id_tensor())
        outs = bass2jax._bass_exec_p.bind(
            *operands, out_avals=tuple(out_avals),
            in_names=tuple(in_names_all), out_names=tuple(out_names),
            lowering_input_output_aliases=(),
            sim_require_finite=True, sim_require_nnan=True, nc=nc)
        return tuple(outs)

    donate = tuple(range(n_params, n_params + n_outs))
    devices = jax.devices()[:n_cores]
    mesh = Mesh(np.asarray(devices), ("core",))
    sharded = jax.jit(
        shard_map(_body, mesh=mesh,
                  in_specs=(PartitionSpec("core"),) * (n_params + n_outs),
                  out_specs=(PartitionSpec("core"),) * n_outs,
                  check_rep=False),
        donate_argnums=donate, keep_unused=True)

    def run_fn(in_maps):
        import jax
        per_core = [[np.asarray(m[nm]) for nm in in_names] for m in in_maps]
        concat_in = [
            np.concatenate([per_core[c][i] for c in range(n_cores)], axis=0)
            for i in range(n_params)]
        concat_zeros = [
            np.zeros((n_cores * z.shape[0], *z.shape[1:]), z.dtype)
            for z in zero_outs]
        out_arrs = sharded(*concat_in, *concat_zeros)
        jax.block_until_ready(out_arrs)
        return [
            {nm: np.asarray(out_arrs[i]).reshape(n_cores, *out_avals[i].shape)[c]
             for i, nm in enumerate(out_names)}
            for c in range(n_cores)]

    return run_fn


def kernel(signal, patches_idx, conv_kernel, kernel_weights, biases):
    with_bias = bool(np.any(biases))
    key = ("k", with_bias)
    if key not in _CACHE:
        nc = _build_nc(with_bias)
        _CACHE[key] = (nc, _make_runner(nc))
    nc, run = _CACHE[key]

    in_maps = []
    for b in range(B):
        m = _prep_inputs_core(b, signal, patches_idx, conv_kernel,
                              kernel_weights, biases)
        in_maps.append(m)

    results = run(in_maps)
    out = np.stack([results[b]["outd"] for b in range(B)], axis=0)
    return out.astype(np.float32)


# revision 27
# speedup vs baseline: 1.7794x; 1.0496x over previous
"""Trainium2 Bass kernel for nn_BinaryTreeShInvariantConv.

Per (b, v): gather P=32 neighbor rows of signal[b] (Cin=64), contract over P
against conv_kernel[b,v] -> y[Cin, R*N], square, sum SH orders per degree l,
sqrt(+eps), contract [Cin*R*(L+1)=512] against kernel_weights -> [Cout=128],
bias + relu.

Sharding: data-parallel over batch B=8 -> one batch per NeuronCore (SPMD).

Design (driven by the CoreSim v1 cost model, which prices each instruction
as free-size x engine-cycle charged serially to its issuing engine):
  - Gather reads bf16 rows PACKED AS uint64 (16 u64 = 64 bf16 channels):
    the gather is priced as a generic Pool op at out-free-ELEMENTS x 0.83ns,
    so 8x fewer elements -> 4 ops x 427ns per 512-v supergroup (13.7us total
    vs 218us naive).
  - "Pair-diagonal" lhsT: gathered rows land directly in block-diagonal
    [64 part, 128 col] bf16 slabs (2 v's per slab; off-diag zeros memset
    once per buffer). MM1 -> [128 part = (v-parity, c), 32 rn] per pair:
    half the PE columns of a 4-v block-diag rhs; conv_kernel needs one
    [128, 4096] DMA per supergroup.
  - Degree sums as strided bf16 tensor_tensor adds (2x DVE mode) instead of
    reduce_sum (no fast mode).
  - Squares (PSUM f32 -> bf16) split between ACT (activation Square) and
    Pool (tensor_tensor mult) to balance engine occupancy.
  - MM3 packs 128 v-pairs in output partitions.

Supergroup = 512 v's (NSG=8): 4 gather ops (one per lhsT quadrant class,
4096 idxs each), 1 kre DMA, 8x32 MM1 matmuls -> 8 PSUM granules -> squares
-> per 256-v block: DVE degree-adds, ACT sqrt(+eps), MM3 (2 par x 8 rl
accumulating), DVE relu, store.
"""

import sys

sys.path.insert(0, "/opt/trn_rl_repo")

import numpy as np

import concourse.bacc as bacc
import concourse.mybir as mybir
import concourse.tile as tile
from concourse import ap_utils
from concourse.bass import MemorySpace
from concourse import bass2jax

B, V, P, CIN, R, COUT = 8, 4096, 32, 64, 2, 128
NSH, NDEG = 16, 4
VSG = 512            # v's per supergroup
NSG = V // VSG       # 8 supergroups
NSLAB = 128          # v-pairs per part-block (h) per supergroup
# engine per square granule (gidx = 4h+gl), by sg parity: balance ACT/Pool/DVE
SQ_ENG = [list("PDADPDAP"), list("DPADAPAD")]
BF16 = mybir.dt.bfloat16
F32 = mybir.dt.float32
I16 = mybir.dt.int16
U64 = mybir.dt.uint64

_CACHE = {}
_SKIP = set()  # debug: subset of {'gather','mm1','post','mm3','store'}


def _dma_gather_any(eng, out_ap, in_ap, idxs_ap, num_idxs, elem_size,
                    single_packet=True, nreg=None):
    """bass.dma_gather with relaxed asserts. The executor flattens the out AP
    and reshapes it to (128, ceil(n/128), elem) as a VIEW: row i lands at
    flat slot (i%128)*chunks + i//128. The AP's stride tree must therefore be
    numpy-reshape-compatible with that shape (seamless merges)."""
    assert idxs_ap.dtype == I16
    assert in_ap.space == MemorySpace.DRAM
    assert in_ap.dtype == out_ap.dtype
    elem_step = in_ap.ap[0][0]
    stride_bytes = elem_step * mybir.dt.size(in_ap.dtype)
    assert stride_bytes % 256 == 0 and stride_bytes // 256 < 256
    assert ap_utils.ap_is_contiguous(idxs_ap.ap[1:])
    assert in_ap.ap[-1][1] == elem_size
    total = 1
    for _, c in out_ap.ap:
        total *= c
    assert total == ((num_idxs + 127) // 128) * 128 * elem_size

    _in_ap = eng.lower_ap_dma(in_ap, for_custom_bir_dma=True)
    if nreg is None:
        nreg = eng.to_reg(num_idxs)
    return eng.add_instruction(
        mybir.InstDMAGatherAnt(
            name=eng.bass.get_next_instruction_name(),
            ins=[*_in_ap, eng.lower_ap(idxs_ap),
                 eng.lower_val_access(nreg)],
            outs=[eng.lower_ap(out_ap)],
            transpose=False,
            num_idxs=num_idxs,
            elem_size=elem_size,
            stride_bytes_256=stride_bytes // 256,
            gen_mode=0,
            single_packet=single_packet,
            queue_num=0,
            sbuf_tokens_per_rank=0,
            sbuf_free_dim_per_rank=0,
            sbuf_free_dim_pad_per_rank=0,
            sbuf_byte_offset=0,
        ))


def _build_nc(with_bias):
    nc = bacc.Bacc("TRN2", target_bir_lowering=False, debug=False,
                   enable_asserts=False, dynamic_dma_scratch_size=32768)
    AF = mybir.ActivationFunctionType
    ALU = mybir.AluOpType

    sig = nc.dram_tensor("sig", [V, 32], U64, kind="ExternalInput")
    kre = nc.dram_tensor("kre", [NSG, 128, NSLAB, 2 * NSH], BF16,
                         kind="ExternalInput")
    idx = nc.dram_tensor("idx", [128, V * P // 16], I16, kind="ExternalInput")
    wsb = nc.dram_tensor("wsb", [128, 8 * COUT], BF16, kind="ExternalInput")
    bia = nc.dram_tensor("bia", [1, COUT], F32, kind="ExternalInput")
    zz = nc.dram_tensor("zz", [128, NSLAB * 128], BF16, kind="ExternalInput")
    outd = nc.dram_tensor("outd", [V, COUT], F32, kind="ExternalOutput")

    with tile.TileContext(nc) as tc:
        with (
            tc.tile_pool(name="const", bufs=1) as constp,
            tc.tile_pool(name="patbf", bufs=2) as patbfp,
            tc.tile_pool(name="kre", bufs=2) as krep,
            tc.tile_pool(name="ysq", bufs=3) as ysqp,
            tc.tile_pool(name="zsb", bufs=2) as zsbp,
            tc.tile_pool(name="zt", bufs=2) as ztp,
            tc.tile_pool(name="osb", bufs=2) as osbp,
            tc.tile_pool(name="ps1", bufs=3, space="PSUM") as ps1p,
            tc.tile_pool(name="ps3", bufs=2, space="PSUM") as ps3p,
        ):
            w_t = constp.tile([128, 8 * COUT], BF16, tag="w")
            nc.sync.dma_start(w_t[:], wsb.ap())
            idx_t = constp.tile([128, V * P // 16], I16, tag="idx")
            nc.sync.dma_start(idx_t[:], idx.ap())
            if with_bias:
                bias_t = constp.tile([1, COUT], F32, tag="bias")
                nc.sync.dma_start(bias_t[:], bia.ap())
            eps_t = constp.tile([128, 1], F32, tag="eps")
            nc.vector.memset(eps_t[:], 1e-4)

            # bf16 patch tiles: zero quadrants persist across supergroups;
            # gathers refill only the data quadrants. Zero buffer 0 via SP
            # DMA (from a zeros DRAM tensor) and buffer 1 on Pool so startup
            # overlaps and the vector engines stay free.
            patbfs = [patbfp.tile([128, NSLAB * 128], BF16, tag="patbf",
                                  name=f"patbf_{i}") for i in range(2)]
            nc.sync.dma_start(patbfs[0][:], zz.ap())
            nc.gpsimd.memset(patbfs[1][:, :].bitcast(F32), 0.0)

            nregs = {}
            for eng in (nc.gpsimd, nc.scalar):
                nregs[id(eng)] = eng.to_reg(VSG * P // 4)

            for sg in range(NSG):
                sgb = VSG * sg
                # --- gathers: 4 quadrant ops, 4096 idxs each ---------------
                patbf = patbfs[sg % 2]
                # u64 view: [p][b 32][e 4][blk 2][c8 16]; slab s=32*(i%4)+i//128
                p8v = patbf[:, :].bitcast(U64).rearrange(
                    "p (bb e blk c) -> p bb e blk c", bb=32, e=4, blk=2)
                geng = ([nc.gpsimd, nc.gpsimd, nc.gpsimd, nc.scalar]
                        if sg % 2 == 0 else
                        [nc.gpsimd, nc.scalar, nc.gpsimd, nc.scalar])
                for op in range(4 if 'gather' not in _SKIP else 0):
                    h, par = op // 2, op % 2
                    gv = p8v[64 * h + 32 * par:64 * h + 32 * par + 32,
                             :, :, par, :]
                    col0 = (V * P // 16) // NSG * sg + 256 * op
                    eng = geng[op]
                    _dma_gather_any(
                        eng, gv, sig.ap()[:, 0:CIN // 4],
                        idx_t[:, col0:col0 + 256], VSG * P // 4, CIN // 4,
                        single_packet=False, nreg=nregs[id(eng)])

                # --- kre load ----------------------------------------------
                kre_t = krep.tile([128, NSLAB * 2 * NSH], BF16, tag="kre")
                nc.sync.dma_start(kre_t[:], kre.ap()[sg])

                # --- per part-block h: MM1 granules + post + MM3 -----------
                for h in range(2):
                    ysq = ysqp.tile([128, 4 * 32 * 32], BF16, tag="ysq")
                    for gl in range(4):
                        ps1 = ps1p.tile([128, 1024], F32, tag="ps1")
                        for ql in range(32 if 'mm1' not in _SKIP else 0):
                            s = 32 * gl + ql
                            nc.tensor.matmul(
                                ps1[:, 32 * ql:32 * ql + 32],
                                patbf[64 * h:64 * h + 64,
                                      128 * s:128 * s + 128],
                                kre_t[64 * h:64 * h + 64,
                                      32 * s:32 * s + 32],
                                start=True, stop=True)
                        if 'post' not in _SKIP:
                            dst = ysq[:, 1024 * gl:1024 * (gl + 1)]
                            se = SQ_ENG[sg % 2][4 * h + gl]
                            if se == 'A':
                                nc.scalar.activation(dst, ps1[:], AF.Square)
                            else:
                                e = nc.gpsimd if se == 'P' else nc.vector
                                e.tensor_tensor(dst, ps1[:], ps1[:], ALU.mult)

                    # ysq: [128, (g 4, pl 32, n 16, r 2)]
                    yv = ysq[:, :].rearrange("p (g pl n r) -> p g pl n r",
                                             g=4, pl=32, n=NSH)
                    # n split (n2, two): n = 2*n2 + two, for stride-2 windows
                    yv2 = ysq[:, :].rearrange(
                        "p (g pl n2 two r) -> p g pl n2 two r",
                        g=4, pl=32, n2=NSH // 2, two=2)
                    # zsb: [128, (g 4, pl 32, l 4, r 2)]
                    zsb = zsbp.tile([128, 4 * 32 * NDEG * 2], BF16, tag="zsb")
                    zv = zsb[:, :].rearrange("p (g pl l r) -> p g pl l r",
                                             g=4, pl=32, l=NDEG)
                    # zt scratch: [128, (g 4, pl 32, t 5, r 2)]
                    zt = ztp.tile([128, 4 * 32 * 5 * 2], BF16, tag="zt")
                    tv = zt[:, :].rearrange("p (g pl t r) -> p g pl t r",
                                            g=4, pl=32, t=5)
                    TT = nc.vector.tensor_tensor
                    if 'post' not in _SKIP:
                        # l=3: n 9..15 (7 terms): {9,11,13}+{10,12,14}, +15
                        TT(tv[:, :, :, 0:3, :], yv2[:, :, :, 4:7, 1, :],
                           yv2[:, :, :, 5:8, 0, :], ALU.add)
                        TT(zv[:, :, :, 3, :], tv[:, :, :, 0, :],
                           tv[:, :, :, 1, :], ALU.add)
                        TT(zv[:, :, :, 3, :], zv[:, :, :, 3, :],
                           tv[:, :, :, 2, :], ALU.add)
                        TT(zv[:, :, :, 3, :], zv[:, :, :, 3, :],
                           yv2[:, :, :, 7, 1, :], ALU.add)
                        # l=2: n 4..8 (5 terms): {4,6}+{5,7}, +8
                        TT(tv[:, :, :, 3:5, :], yv2[:, :, :, 2:4, 0, :],
                           yv2[:, :, :, 2:4, 1, :], ALU.add)
                        TT(zv[:, :, :, 2, :], tv[:, :, :, 3, :],
                           tv[:, :, :, 4, :], ALU.add)
                        TT(zv[:, :, :, 2, :], zv[:, :, :, 2, :],
                           yv2[:, :, :, 4, 0, :], ALU.add)
                        # l=1: n 1..3
                        TT(zv[:, :, :, 1, :], yv2[:, :, :, 0, 1, :],
                           yv2[:, :, :, 1, 0, :], ALU.add)
                        TT(zv[:, :, :, 1, :], zv[:, :, :, 1, :],
                           yv2[:, :, :, 1, 1, :], ALU.add)
                        # sqrt(x + eps): l=0 from ysq n=0; l>=1 in place
                        nc.scalar.activation(zv[:, :, :, 0, :],
                                             yv[:, :, :, 0, :], AF.Sqrt,
                                             bias=eps_t[:])
                        nc.scalar.activation(zv[:, :, :, 1:4, :],
                                             zv[:, :, :, 1:4, :], AF.Sqrt,
                                             bias=eps_t[:])

                    # --- MM3: out[pair, (par, i)] --------------------------
                    ps3 = ps3p.tile([128, 2 * COUT], F32, tag="ps3")
                    for par in range(2 if 'mm3' not in _SKIP else 0):
                        for rl in range(8):
                            lhsT = zv[64 * par:64 * par + 64, :, :, rl // 2,
                                      rl % 2]
                            rhs = w_t[64 * par:64 * par + 64,
                                      COUT * rl:COUT * (rl + 1)]
                            nc.tensor.matmul(
                                ps3[:, COUT * par:COUT * (par + 1)],
                                lhsT, rhs, start=(rl == 0), stop=(rl == 7),
                                skip_group_check=True)

                    # --- relu (+bias) + store ------------------------------
                    osb = osbp.tile([128, 2 * COUT], F32, tag="osb")
                    if with_bias:
                        for par in range(2):
                            nc.vector.tensor_add(
                                osb[:, COUT * par:COUT * (par + 1)],
                                ps3[:, COUT * par:COUT * (par + 1)],
                                bias_t[:, :].broadcast(0, 128))
                        nc.scalar.activation(osb[:], osb[:], AF.Relu)
                    else:
                        nc.gpsimd.tensor_scalar_max(osb[:], ps3[:], 0.0)
                    if 'store' not in _SKIP:
                        dst = outd.ap()[sgb + 256 * h:sgb + 256 * (h + 1), :]
                        dst = dst.rearrange("(pl par) i -> pl par i", par=2)
                        nc.sync.dma_start(
                            dst, osb[:, :].rearrange("p (par i) -> p par i",
                                                     par=2))

    nc.compile()
    return nc


# map (p_pt, s) -> gather list position i (see _dma_gather_any docstring)
_SS, _PP = np.meshgrid(np.arange(NSLAB), np.arange(32), indexing='ij')
_IPOS = (128 * (_SS % 32) + 4 * _PP + _SS // 32).ravel()


def _prep_inputs_core(b, signal, patches_idx, conv_kernel, kernel_weights,
                      biases):
    bf = mybir.dt.np(BF16)
    sigrow = np.zeros((V, 128), dtype=bf)
    sigrow[:, 0:CIN] = signal[b].astype(bf)
    sig_u64 = np.ascontiguousarray(sigrow).view(np.uint64)   # [V, 32]

    pidx = patches_idx[b, :, :, 1]
    krn = conv_kernel[b].transpose(0, 1, 3, 2)              # [V, P, NSH, R]
    # kre[sg, 64h+32par+p, s, n*2+r] = krn[sg*512+256h+2s+par, p]
    k6 = krn.reshape(NSG, 2, NSLAB, 2, P, 2 * NSH)
    kre_ = np.ascontiguousarray(k6.transpose(0, 1, 3, 4, 2, 5)).reshape(
        NSG, 128, NSLAB, 2 * NSH).astype(bf)

    # idx: per sg, 4 ops x 4096 idxs; arr[i] = pidx[v(h,par,s), p_pt]
    # (rows 16..127 are zero padding -- the gather ucode reads rows 0:16;
    # loading a full-height tensor avoids an uninitialized-SBUF memset)
    idxh = np.zeros((128, V * P // 16), dtype=np.int16)
    for sg in range(NSG):
        for op in range(4):
            h, par = op // 2, op % 2
            vv = VSG * sg + 256 * h + 2 * _SS + par
            mat = pidx[vv.ravel(), _PP.ravel()].astype(np.int16)
            arr = np.empty(VSG * P // 4, dtype=np.int16)
            arr[_IPOS] = mat
            col0 = (V * P // 16) // NSG * sg + 256 * op
            idxh[:16, col0:col0 + 256] = arr.reshape(256, 16).T

    w = kernel_weights.transpose(1, 3, 2, 0).reshape(CIN, 8 * COUT)
    wsb = np.concatenate([w, w], axis=0).astype(bf)
    bia = biases.reshape(1, COUT).astype(np.float32)
    zz = np.zeros((128, NSLAB * 128), dtype=bf)
    return {"sig": sig_u64, "kre": kre_, "idx": idxh, "wsb": wsb, "bia": bia,
            "zz": zz}


def _make_runner(nc, n_cores=8):
    import jax
    from jax.sharding import Mesh, PartitionSpec
    from jax.experimental.shard_map import shard_map

    bass2jax.install_neuronx_cc_hook()
    partition_name = (nc.partition_id_tensor.name
                      if nc.partition_id_tensor else None)
    in_names, out_names, out_avals, zero_outs = [], [], [], []
    for alloc in nc.m.functions[0].allocations:
        if not isinstance(alloc, mybir.MemoryLocationSet):
            continue
        name = alloc.memorylocations[0].name
        if alloc.kind == "ExternalInput":
            if name != partition_name:
                in_names.append(name)
        elif alloc.kind == "ExternalOutput":
            out_names.append(name)
            shape = tuple(alloc.tensor_shape)
            dtype = mybir.dt.np(alloc.dtype)
            out_avals.append(jax.core.ShapedArray(shape, dtype))
            zero_outs.append(np.zeros(shape, dtype))
    n_params, n_outs = len(in_names), len(out_avals)
    in_names_all = list(in_names) + list(out_names)
    if partition_name is not None:
        in_names_all.append(partition_name)

    def _body(*args):
        operands = list(args)
        if partition_name is not None:
            operands.append(bass2jax.partition_id_tensor())
        outs = bass2jax._bass_exec_p.bind(
            *operands, out_avals=tuple(out_avals),
            in_names=tuple(in_names_all), out_names=tuple(out_names),
            lowering_input_output_aliases=(),
            sim_require_finite=True, sim_require_nnan=True, nc=nc)
        return tuple(outs)

    donate = tuple(range(n_params, n_params + n_outs))
    devices = jax.devices()[:n_cores]
    mesh = Mesh(np.asarray(devices), ("core",))
    sharded = jax.jit(
        shard_map(_body, mesh=mesh,
                  in_specs=(PartitionSpec("core"),) * (n_params + n_outs),
                  out_specs=(PartitionSpec("core"),) * n_outs,
                  check_rep=False),
        donate_argnums=donate, keep_unused=True)

    def run_fn(in_maps):
        import jax
        per_core = [[np.asarray(m[nm]) for nm in in_names] for m in in_maps]
        concat_in = [
            np.concatenate([per_core[c][i] for c in range(n_cores)], axis=0)
            for i in range(n_params)]
        concat_zeros = [
            np.zeros((n_cores * z.shape[0], *z.shape[1:]), z.dtype)
            for z in zero_outs]
        out_arrs = sharded(*concat_in, *concat_zeros)
        jax.block_until_ready(out_arrs)
        return [
            {nm: np.asarray(out_arrs[i]).reshape(n_cores, *out_avals[i].shape)[c]
             for i, nm in enumerate(out_names)}
            for c in range(n_cores)]

    return run_fn


def kernel(signal, patches_idx, conv_kernel, kernel_weights, biases):
    with_bias = bool(np.any(biases))
    key = ("k", with_bias)
    if key not in _CACHE:
        nc = _build_nc(with_bias)
        _CACHE[key] = (nc, _make_runner(nc))
    nc, run = _CACHE[key]

    in_maps = []
    for b in range(B):
        m = _prep_inputs_core(b, signal, patches_idx, conv_kernel,
                              kernel_weights, biases)
        in_maps.append(m)

    results = run(in_maps)
    out = np.stack([results[b]["outd"] for b in range(B)], axis=0)
    return out.astype(np.float32)


# revision 32
# speedup vs baseline: 2.0380x; 1.1453x over previous
"""Trainium2 Bass kernel for nn_BinaryTreeShInvariantConv.

Per (b, v): gather P=32 neighbor rows of signal[b] (Cin=64), contract over P
against conv_kernel[b,v] -> y[Cin, R*N], square, sum SH orders per degree l,
sqrt(+eps), contract [Cin*R*(L+1)=512] against kernel_weights -> [Cout=128],
bias + relu.

Sharding: data-parallel over batch B=8 -> one batch per NeuronCore (SPMD).

Design (driven by the CoreSim v1 cost model, which prices each instruction
as free-size x engine-cycle charged serially to its issuing engine):
  - Gather reads bf16 rows PACKED AS uint64 (16 u64 = 64 bf16 channels):
    the gather is priced as a generic Pool op at out-free-ELEMENTS x 0.83ns,
    so 8x fewer elements -> 4 ops x 427ns per 512-v supergroup (13.7us total
    vs 218us naive).
  - "Pair-diagonal" lhsT: gathered rows land directly in block-diagonal
    [64 part, 128 col] bf16 slabs (2 v's per slab; off-diag zeros memset
    once per buffer). MM1 -> [128 part = (v-parity, c), 32 rn] per pair:
    half the PE columns of a 4-v block-diag rhs; conv_kernel needs one
    [128, 4096] DMA per supergroup.
  - Degree sums as strided bf16 tensor_tensor adds (2x DVE mode) instead of
    reduce_sum (no fast mode).
  - Squares (PSUM f32 -> bf16) split between ACT (activation Square) and
    Pool (tensor_tensor mult) to balance engine occupancy.
  - MM3 packs 128 v-pairs in output partitions.

Supergroup = 512 v's (NSG=8): 4 gather ops (one per lhsT quadrant class,
4096 idxs each), 1 kre DMA, 8x32 MM1 matmuls -> 8 PSUM granules -> squares
-> per 256-v block: DVE degree-adds, ACT sqrt(+eps), MM3 (2 par x 8 rl
accumulating), DVE relu, store.
"""

import sys

sys.path.insert(0, "/opt/trn_rl_repo")

import numpy as np

import concourse.bacc as bacc
import concourse.mybir as mybir
import concourse.tile as tile
from concourse import ap_utils
from concourse.bass import MemorySpace
from concourse import bass2jax

B, V, P, CIN, R, COUT = 8, 4096, 32, 64, 2, 128
NSH, NDEG = 16, 4
VSG = 512            # v's per supergroup
NSG = V // VSG       # 8 supergroups
NSLAB = 128          # v-pairs per part-block (h) per supergroup
# engine per square granule (gidx = 4h+gl), by sg parity: balance ACT/Pool/DVE
SQ_ENG = [list("PDADPDAP"), list("DPADAPAD")]
BF16 = mybir.dt.bfloat16
F32 = mybir.dt.float32
I16 = mybir.dt.int16
U64 = mybir.dt.uint64

_CACHE = {}
_SKIP = set()  # debug: subset of {'gather','mm1','post','mm3','store'}


def _dma_gather_any(eng, out_ap, in_ap, idxs_ap, num_idxs, elem_size,
                    single_packet=True, nreg=None):
    """bass.dma_gather with relaxed asserts. The executor flattens the out AP
    and reshapes it to (128, ceil(n/128), elem) as a VIEW: row i lands at
    flat slot (i%128)*chunks + i//128. The AP's stride tree must therefore be
    numpy-reshape-compatible with that shape (seamless merges)."""
    assert idxs_ap.dtype == I16
    assert in_ap.space == MemorySpace.DRAM
    assert in_ap.dtype == out_ap.dtype
    elem_step = in_ap.ap[0][0]
    stride_bytes = elem_step * mybir.dt.size(in_ap.dtype)
    assert stride_bytes % 256 == 0 and stride_bytes // 256 < 256
    assert ap_utils.ap_is_contiguous(idxs_ap.ap[1:])
    assert in_ap.ap[-1][1] == elem_size
    total = 1
    for _, c in out_ap.ap:
        total *= c
    assert total == ((num_idxs + 127) // 128) * 128 * elem_size

    _in_ap = eng.lower_ap_dma(in_ap, for_custom_bir_dma=True)
    if nreg is None:
        nreg = eng.to_reg(num_idxs)
    return eng.add_instruction(
        mybir.InstDMAGatherAnt(
            name=eng.bass.get_next_instruction_name(),
            ins=[*_in_ap, eng.lower_ap(idxs_ap),
                 eng.lower_val_access(nreg)],
            outs=[eng.lower_ap(out_ap)],
            transpose=False,
            num_idxs=num_idxs,
            elem_size=elem_size,
            stride_bytes_256=stride_bytes // 256,
            gen_mode=0,
            single_packet=single_packet,
            queue_num=0,
            sbuf_tokens_per_rank=0,
            sbuf_free_dim_per_rank=0,
            sbuf_free_dim_pad_per_rank=0,
            sbuf_byte_offset=0,
        ))


def _build_nc(with_bias):
    nc = bacc.Bacc("TRN2", target_bir_lowering=False, debug=False,
                   enable_asserts=False, dynamic_dma_scratch_size=32768)
    AF = mybir.ActivationFunctionType
    ALU = mybir.AluOpType

    sig = nc.dram_tensor("sig", [V, 32], U64, kind="ExternalInput")
    kre = nc.dram_tensor("kre", [NSG, 128, NSLAB, 2 * NSH], BF16,
                         kind="ExternalInput")
    idx = nc.dram_tensor("idx", [128, V * P // 16], I16, kind="ExternalInput")
    wsb = nc.dram_tensor("wsb", [128, 8 * COUT], BF16, kind="ExternalInput")
    bia = nc.dram_tensor("bia", [1, COUT], F32, kind="ExternalInput")
    outd = nc.dram_tensor("outd", [V, COUT], F32, kind="ExternalOutput")

    with tile.TileContext(nc) as tc:
        with (
            tc.tile_pool(name="const", bufs=1) as constp,
            tc.tile_pool(name="patbf", bufs=2) as patbfp,
            tc.tile_pool(name="kre", bufs=3) as krep,
            tc.tile_pool(name="ysq", bufs=3) as ysqp,
            tc.tile_pool(name="zsb", bufs=2) as zsbp,
            tc.tile_pool(name="zt", bufs=2) as ztp,
            tc.tile_pool(name="osb", bufs=2) as osbp,
            tc.tile_pool(name="ps1", bufs=3, space="PSUM") as ps1p,
            tc.tile_pool(name="ps3", bufs=2, space="PSUM") as ps3p,
        ):
            idx_t = constp.tile([128, V * P // 16], I16, tag="idx")
            ISG = (V * P // 16) // NSG
            nc.sync.dma_start(idx_t[:, 0:ISG], idx.ap()[:, 0:ISG])
            nc.sync.dma_start(idx_t[:, ISG:], idx.ap()[:, ISG:])
            w_t = constp.tile([128, 8 * COUT], BF16, tag="w")
            nc.sync.dma_start(w_t[:], wsb.ap())
            if with_bias:
                bias_t = constp.tile([1, COUT], F32, tag="bias")
                nc.sync.dma_start(bias_t[:], bia.ap())
            eps_t = constp.tile([128, 1], F32, tag="eps")
            nc.vector.memset(eps_t[:], 1e-4)

            # bf16 patch tiles: zero quadrants persist across supergroups;
            # gathers refill only the data quadrants. Buffer 0 zeroed on DVE
            # (idle at startup) and buffer 1 on Pool (fits before its first
            # gather, which waits on buffer 0 anyway).
            patbfs = [patbfp.tile([128, NSLAB * 128], BF16, tag="patbf",
                                  name=f"patbf_{i}") for i in range(2)]
            nc.vector.memset(patbfs[0][:, :].bitcast(F32), 0.0)
            nc.gpsimd.memset(patbfs[1][:, :].bitcast(F32), 0.0)

            nregs = {}
            for eng in (nc.gpsimd, nc.scalar):
                nregs[id(eng)] = eng.to_reg(VSG * P // 4)

            for sg in range(NSG):
                sgb = VSG * sg
                # --- gathers: 4 quadrant ops, 4096 idxs each ---------------
                patbf = patbfs[sg % 2]
                # u64 view: [p][b 32][e 4][blk 2][c8 16]; slab s=32*(i%4)+i//128
                p8v = patbf[:, :].bitcast(U64).rearrange(
                    "p (bb e blk c) -> p bb e blk c", bb=32, e=4, blk=2)
                geng = ([nc.gpsimd, nc.gpsimd, nc.gpsimd, nc.scalar]
                        if sg % 2 == 0 else
                        [nc.gpsimd, nc.scalar, nc.gpsimd, nc.scalar])
                for op in range(4 if 'gather' not in _SKIP else 0):
                    h, par = op // 2, op % 2
                    gv = p8v[64 * h + 32 * par:64 * h + 32 * par + 32,
                             :, :, par, :]
                    col0 = (V * P // 16) // NSG * sg + 256 * op
                    eng = geng[op]
                    _dma_gather_any(
                        eng, gv, sig.ap()[:, 0:CIN // 4],
                        idx_t[:, col0:col0 + 256], VSG * P // 4, CIN // 4,
                        single_packet=False, nreg=nregs[id(eng)])

                # --- kre load ----------------------------------------------
                kre_t = krep.tile([128, NSLAB * 2 * NSH], BF16, tag="kre")
                nc.sync.dma_start(kre_t[:], kre.ap()[sg])

                # --- per part-block h: MM1 granules + post + MM3 -----------
                for h in range(2):
                    ysq = ysqp.tile([128, 4 * 32 * 32], BF16, tag="ysq")
                    for gl in range(4):
                        ps1 = ps1p.tile([128, 1024], F32, tag="ps1")
                        for ql in range(32 if 'mm1' not in _SKIP else 0):
                            s = 32 * gl + ql
                            nc.tensor.matmul(
                                ps1[:, 32 * ql:32 * ql + 32],
                                patbf[64 * h:64 * h + 64,
                                      128 * s:128 * s + 128],
                                kre_t[64 * h:64 * h + 64,
                                      32 * s:32 * s + 32],
                                start=True, stop=True)
                        if 'post' not in _SKIP:
                            dst = ysq[:, 1024 * gl:1024 * (gl + 1)]
                            se = SQ_ENG[sg % 2][4 * h + gl]
                            if se == 'A':
                                nc.scalar.activation(dst, ps1[:], AF.Square)
                            else:
                                e = nc.gpsimd if se == 'P' else nc.vector
                                e.tensor_tensor(dst, ps1[:], ps1[:], ALU.mult)

                    # ysq: [128, (g 4, pl 32, n 16, r 2)]
                    yv = ysq[:, :].rearrange("p (g pl n r) -> p g pl n r",
                                             g=4, pl=32, n=NSH)
                    # n split (n2, two): n = 2*n2 + two, for stride-2 windows
                    yv2 = ysq[:, :].rearrange(
                        "p (g pl n2 two r) -> p g pl n2 two r",
                        g=4, pl=32, n2=NSH // 2, two=2)
                    # zsb: [128, (g 4, pl 32, l 4, r 2)]
                    zsb = zsbp.tile([128, 4 * 32 * NDEG * 2], BF16, tag="zsb")
                    zv = zsb[:, :].rearrange("p (g pl l r) -> p g pl l r",
                                             g=4, pl=32, l=NDEG)
                    # zt scratch: [128, (g 4, pl 32, t 5, r 2)]
                    zt = ztp.tile([128, 4 * 32 * 5 * 2], BF16, tag="zt")
                    tv = zt[:, :].rearrange("p (g pl t r) -> p g pl t r",
                                            g=4, pl=32, t=5)
                    TT = nc.vector.tensor_tensor
                    if 'post' not in _SKIP:
                        # l=3: n 9..15 (7 terms): {9,11,13}+{10,12,14}, +15
                        TT(tv[:, :, :, 0:3, :], yv2[:, :, :, 4:7, 1, :],
                           yv2[:, :, :, 5:8, 0, :], ALU.add)
                        TT(zv[:, :, :, 3, :], tv[:, :, :, 0, :],
                           tv[:, :, :, 1, :], ALU.add)
                        TT(zv[:, :, :, 3, :], zv[:, :, :, 3, :],
                           tv[:, :, :, 2, :], ALU.add)
                        TT(zv[:, :, :, 3, :], zv[:, :, :, 3, :],
                           yv2[:, :, :, 7, 1, :], ALU.add)
                        # l=2: n 4..8 (5 terms): {4,6}+{5,7}, +8
                        TT(tv[:, :, :, 3:5, :], yv2[:, :, :, 2:4, 0, :],
                           yv2[:, :, :, 2:4, 1, :], ALU.add)
                        TT(zv[:, :, :, 2, :], tv[:, :, :, 3, :],
                           tv[:, :, :, 4, :], ALU.add)
                        TT(zv[:, :, :, 2, :], zv[:, :, :, 2, :],
                           yv2[:, :, :, 4, 0, :], ALU.add)
                        # l=1: n 1..3
                        TT(zv[:, :, :, 1, :], yv2[:, :, :, 0, 1, :],
                           yv2[:, :, :, 1, 0, :], ALU.add)
                        TT(zv[:, :, :, 1, :], zv[:, :, :, 1, :],
                           yv2[:, :, :, 1, 1, :], ALU.add)
                        # sqrt(x + eps): l=0 from ysq n=0; l>=1 in place
                        nc.scalar.activation(zv[:, :, :, 0, :],
                                             yv[:, :, :, 0, :], AF.Sqrt,
                                             bias=eps_t[:])
                        nc.scalar.activation(zv[:, :, :, 1:4, :],
                                             zv[:, :, :, 1:4, :], AF.Sqrt,
                                             bias=eps_t[:])

                    # --- MM3: out[pair, (par, i)] --------------------------
                    ps3 = ps3p.tile([128, 2 * COUT], F32, tag="ps3")
                    for par in range(2 if 'mm3' not in _SKIP else 0):
                        for rl in range(8):
                            lhsT = zv[64 * par:64 * par + 64, :, :, rl // 2,
                                      rl % 2]
                            rhs = w_t[64 * par:64 * par + 64,
                                      COUT * rl:COUT * (rl + 1)]
                            nc.tensor.matmul(
                                ps3[:, COUT * par:COUT * (par + 1)],
                                lhsT, rhs, start=(rl == 0), stop=(rl == 7),
                                skip_group_check=True)

                    # --- relu (+bias) + store ------------------------------
                    osb = osbp.tile([128, 2 * COUT], F32, tag="osb")
                    if with_bias:
                        for par in range(2):
                            nc.vector.tensor_add(
                                osb[:, COUT * par:COUT * (par + 1)],
                                ps3[:, COUT * par:COUT * (par + 1)],
                                bias_t[:, :].broadcast(0, 128))
                        nc.scalar.activation(osb[:], osb[:], AF.Relu)
                    else:
                        nc.gpsimd.tensor_scalar_max(osb[:], ps3[:], 0.0)
                    if 'store' not in _SKIP:
                        dst = outd.ap()[sgb + 256 * h:sgb + 256 * (h + 1), :]
                        dst = dst.rearrange("(pl par) i -> pl par i", par=2)
                        nc.sync.dma_start(
                            dst, osb[:, :].rearrange("p (par i) -> p par i",
                                                     par=2))

    nc.compile()
    return nc


# map (p_pt, s) -> gather list position i (see _dma_gather_any docstring)
_SS, _PP = np.meshgrid(np.arange(NSLAB), np.arange(32), indexing='ij')
_IPOS = (128 * (_SS % 32) + 4 * _PP + _SS // 32).ravel()


def _prep_inputs_core(b, signal, patches_idx, conv_kernel, kernel_weights,
                      biases):
    bf = mybir.dt.np(BF16)
    sigrow = np.zeros((V, 128), dtype=bf)
    sigrow[:, 0:CIN] = signal[b].astype(bf)
    sig_u64 = np.ascontiguousarray(sigrow).view(np.uint64)   # [V, 32]

    pidx = patches_idx[b, :, :, 1]
    krn = conv_kernel[b].transpose(0, 1, 3, 2)              # [V, P, NSH, R]
    # kre[sg, 64h+32par+p, s, n*2+r] = krn[sg*512+256h+2s+par, p]
    k6 = krn.reshape(NSG, 2, NSLAB, 2, P, 2 * NSH)
    kre_ = np.ascontiguousarray(k6.transpose(0, 1, 3, 4, 2, 5)).reshape(
        NSG, 128, NSLAB, 2 * NSH).astype(bf)

    # idx: per sg, 4 ops x 4096 idxs; arr[i] = pidx[v(h,par,s), p_pt]
    # (rows 16..127 are zero padding -- the gather ucode reads rows 0:16;
    # loading a full-height tensor avoids an uninitialized-SBUF memset)
    idxh = np.zeros((128, V * P // 16), dtype=np.int16)
    for sg in range(NSG):
        for op in range(4):
            h, par = op // 2, op % 2
            vv = VSG * sg + 256 * h + 2 * _SS + par
            mat = pidx[vv.ravel(), _PP.ravel()].astype(np.int16)
            arr = np.empty(VSG * P // 4, dtype=np.int16)
            arr[_IPOS] = mat
            col0 = (V * P // 16) // NSG * sg + 256 * op
            idxh[:16, col0:col0 + 256] = arr.reshape(256, 16).T

    w = kernel_weights.transpose(1, 3, 2, 0).reshape(CIN, 8 * COUT)
    wsb = np.concatenate([w, w], axis=0).astype(bf)
    bia = biases.reshape(1, COUT).astype(np.float32)
    return {"sig": sig_u64, "kre": kre_, "idx": idxh, "wsb": wsb, "bia": bia}


def _make_runner(nc, n_cores=8):
    import jax
    from jax.sharding import Mesh, PartitionSpec
    from jax.experimental.shard_map import shard_map

    bass2jax.install_neuronx_cc_hook()
    partition_name = (nc.partition_id_tensor.name
                      if nc.partition_id_tensor else None)
    in_names, out_names, out_avals, zero_outs = [], [], [], []
    for alloc in nc.m.functions[0].allocations:
        if not isinstance(alloc, mybir.MemoryLocationSet):
            continue
        name = alloc.memorylocations[0].name
        if alloc.kind == "ExternalInput":
            if name != partition_name:
                in_names.append(name)
        elif alloc.kind == "ExternalOutput":
            out_names.append(name)
            shape = tuple(alloc.tensor_shape)
            dtype = mybir.dt.np(alloc.dtype)
            out_avals.append(jax.core.ShapedArray(shape, dtype))
            zero_outs.append(np.zeros(shape, dtype))
    n_params, n_outs = len(in_names), len(out_avals)
    in_names_all = list(in_names) + list(out_names)
    if partition_name is not None:
        in_names_all.append(partition_name)

    def _body(*args):
        operands = list(args)
        if partition_name is not None:
            operands.append(bass2jax.partition_id_tensor())
        outs = bass2jax._bass_exec_p.bind(
            *operands, out_avals=tuple(out_avals),
            in_names=tuple(in_names_all), out_names=tuple(out_names),
            lowering_input_output_aliases=(),
            sim_require_finite=True, sim_require_nnan=True, nc=nc)
        return tuple(outs)

    donate = tuple(range(n_params, n_params + n_outs))
    devices = jax.devices()[:n_cores]
    mesh = Mesh(np.asarray(devices), ("core",))
    sharded = jax.jit(
        shard_map(_body, mesh=mesh,
                  in_specs=(PartitionSpec("core"),) * (n_params + n_outs),
                  out_specs=(PartitionSpec("core"),) * n_outs,
                  check_rep=False),
        donate_argnums=donate, keep_unused=True)

    def run_fn(in_maps):
        import jax
        per_core = [[np.asarray(m[nm]) for nm in in_names] for m in in_maps]
        concat_in = [
            np.concatenate([per_core[c][i] for c in range(n_cores)], axis=0)
            for i in range(n_params)]
        concat_zeros = [
            np.zeros((n_cores * z.shape[0], *z.shape[1:]), z.dtype)
            for z in zero_outs]
        out_arrs = sharded(*concat_in, *concat_zeros)
        jax.block_until_ready(out_arrs)
        return [
            {nm: np.asarray(out_arrs[i]).reshape(n_cores, *out_avals[i].shape)[c]
             for i, nm in enumerate(out_names)}
            for c in range(n_cores)]

    return run_fn


def kernel(signal, patches_idx, conv_kernel, kernel_weights, biases):
    with_bias = bool(np.any(biases))
    key = ("k", with_bias)
    if key not in _CACHE:
        nc = _build_nc(with_bias)
        _CACHE[key] = (nc, _make_runner(nc))
    nc, run = _CACHE[key]

    in_maps = []
    for b in range(B):
        m = _prep_inputs_core(b, signal, patches_idx, conv_kernel,
                              kernel_weights, biases)
        in_maps.append(m)

    results = run(in_maps)
    out = np.stack([results[b]["outd"] for b in range(B)], axis=0)
    return out.astype(np.float32)
